# revision 1
# baseline (speedup 1.0000x reference)
"""Transformer-XL multi-head attention on 8 trn2 NeuronCores.

Sharding: tensor-parallel over heads (2 heads/core x 16 heads), all batches on
every core. Host sums the per-core partial output projections.

Per-core pipeline (fp16 operands, fp32 PSUM accumulation):
  1. Projections from host-prepped fp16 XT/posT: kT, quT, qvT, pT in
     [head*d, token] layout; V in [token, head*d] (+ ones column).
  2. Position scores PD[i,t] = (q_i+v). p_t computed unshifted per i-tile,
     copied to fp16 strips in SBUF, written to a DRAM bounce buffer whose
     guard band (t >= 2048) is prefilled with -30000 once at kernel start.
  3. Plain (non-transposing) skewed DMA readback: pos[i, j] = PD[i, j+1023-i]
     comes back at full DMA rate as an affine access pattern; the TXL
     rel-shift and causal mask are applied for free (masked cells land in
     the guard band).
  4. Content scores matmul'd in [j, i] layout; position scores are added
     into the same PSUM accumulation group by identity-rhs transpose matmuls
     (out[j,i] += sum_k pos[k,j] id[k,i]), so no vector adds are needed.
     ACT exp -> unnormalized attn (fp16).
  5. PV matmul with V augmented by a ones column: row 64 of the PSUM
     accumulator is the softmax denominator. Normalize via DVE reciprocal +
     GPSIMD partition broadcast, then the output projection partial (fp16
     out, summed in f32 on host).
"""

import numpy as np

import concourse.bass as bass
from concourse import bacc
import concourse.mybir as mybir
import concourse.tile as tile
from concourse.bass_utils import run_bass_kernel_spmd

F16 = mybir.dt.float16
F32 = mybir.dt.float32
AF = mybir.ActivationFunctionType

CS, PS, BS, DIN, H, D = 1024, 1024, 4, 1024, 16, 64
KS = CS + PS                 # 2048 keys
NCORES = 8
HPC = H // NCORES            # 2 heads per core
TOK = KS * BS                # 8192 kv tokens (b-major: tok = b*KS + j)
QTOK = CS * BS               # 4096 q tokens  (tok = b*CS + i)
TP = KS + 128                # PD row length incl. guard band
GUARD = -30000.0             # large finite fp16 (exp -> 0 after masking)
SCALE = 1.0 / D ** 0.5
NKT = DIN // 128             # 8 contraction tiles of 128

_CACHED = {}


def _included_jts(ib):
    """j-tiles (128 wide) with any unmasked element for i-half ib (512 wide)."""
    i_hi = ib * 512 + 511
    return [jt for jt in range(KS // 128) if jt * 128 <= i_hi + PS]


def _jw(a):
    """valid j-width for i-tile a (i in [128a, 128a+128)): j <= i + PS."""
    return min(KS, PS + (a + 1) * 128)


def build_nc():
    nc = bacc.Bacc()
    xT = nc.declare_dram_parameter("xT", [DIN, TOK], F16, isOutput=False)
    posT = nc.declare_dram_parameter("posT", [DIN, KS], F16, isOutput=False)
    wk = nc.declare_dram_parameter("wk", [DIN, 128], F16, isOutput=False)
    wv = nc.declare_dram_parameter("wv", [DIN, 128], F16, isOutput=False)
    wq = nc.declare_dram_parameter("wq", [DIN, 128], F16, isOutput=False)
    wp = nc.declare_dram_parameter("wp", [DIN, 128], F16, isOutput=False)
    wout = nc.declare_dram_parameter("wout", [128, DIN], F16, isOutput=False)
    ucol = nc.declare_dram_parameter("ucol", [128, 1], F32, isOutput=False)
    vcol = nc.declare_dram_parameter("vcol", [128, 1], F32, isOutput=False)
    ident = nc.declare_dram_parameter("ident", [128, 128], F16, isOutput=False)
    out = nc.declare_dram_parameter("out", [CS, BS, DIN], F16, isOutput=True)

    # PD bounce buffers, one per (b, h, i-half) for fine-grained deps
    pd_dram = [
        [[nc.dram_tensor(f"pd_{b}_{h}_{ib}", [512, TP], F16)
          for ib in range(CS // 512)] for h in range(HPC)]
        for b in range(BS)
    ]

    with tile.TileContext(nc) as tc:
        _body(nc, tc, xT, posT, wk, wv, wq, wp, wout, ucol, vcol, ident, out,
              pd_dram)
    nc.compile()
    return nc


def _body(nc, tc, xT, posT, wk, wv, wq, wp, wout, ucol, vcol, ident, out,
          pd_dram):
    import contextlib

    ctx = contextlib.ExitStack()
    with ctx:
        res = ctx.enter_context(tc.tile_pool(name="res", bufs=1))
        xtp = ctx.enter_context(tc.tile_pool(name="xtp", bufs=int(__import__("os").environ.get("KXTP", "3"))))
        stp = ctx.enter_context(tc.tile_pool(name="stp", bufs=1))
        pop = ctx.enter_context(tc.tile_pool(name="pop", bufs=1))
        import os as _os0
        att = ctx.enter_context(tc.tile_pool(name="att", bufs=int(_os0.environ.get("KATT", "8"))))
        wrk = ctx.enter_context(tc.tile_pool(name="wrk", bufs=int(__import__("os").environ.get("KWRK", "3"))))
        import os as _os
        _pp = [int(c) for c in _os.environ.get("KPSUM", "2221")]
        ps_mm = ctx.enter_context(tc.tile_pool(name="ps_mm", bufs=_pp[0], space="PSUM"))
        ps_pd = ctx.enter_context(tc.tile_pool(name="ps_pd", bufs=_pp[1], space="PSUM"))
        ps_cn = ctx.enter_context(tc.tile_pool(name="ps_cn", bufs=_pp[2], space="PSUM"))
        ps_pv = ctx.enter_context(tc.tile_pool(name="ps_pv", bufs=_pp[3], space="PSUM"))

        # ---- resident tiles
        kT_sb = res.tile([128, TOK], F16)         # [(h,d), b*KS+j]
        quT_sb = res.tile([128, QTOK], F16)       # [(h,d), b*CS+i]
        qvT_sb = res.tile([128, QTOK], F16)
        pT_sb = res.tile([128, KS], F16)
        # V + ones column: slot index = b*HPC*16 + h*16 + jt
        vaug_sb = res.tile([128, BS * HPC * 16, 65], F16)
        outT_sb = res.tile([128, BS, CS], F16)    # [(h,d), b, i]
        wk_sb = res.tile([128, NKT, 128], F16)
        wv_sb = res.tile([128, NKT, 128], F16)
        wq_sb = res.tile([128, NKT, 128], F16)
        wp_sb = res.tile([128, NKT, 128], F16)
        wout_sb = res.tile([128, DIN], F16)
        u_sb = res.tile([128, 1], F32)
        v_sb = res.tile([128, 1], F32)
        id_sb = res.tile([128, 128], F16)

        nc.sync.dma_start(out=wp_sb[:], in_=wp.ap().rearrange("(a p) n -> p a n", p=128))
        nc.sync.dma_start(out=wk_sb[:], in_=wk.ap().rearrange("(a p) n -> p a n", p=128))
        nc.sync.dma_start(out=wv_sb[:], in_=wv.ap().rearrange("(a p) n -> p a n", p=128))
        nc.sync.dma_start(out=wq_sb[:], in_=wq.ap().rearrange("(a p) n -> p a n", p=128))
        nc.sync.dma_start(out=wout_sb[:], in_=wout[:, :])
        nc.sync.dma_start(out=u_sb[:], in_=ucol[:, :])
        nc.sync.dma_start(out=v_sb[:], in_=vcol[:, :])
        nc.sync.dma_start(out=id_sb[:], in_=ident[:, :])
        nc.vector.memset(vaug_sb[:, :, 64:65], 1.0)
        guard_sb = res.tile([128, 512], F16)
        nc.vector.memset(guard_sb[:], GUARD)
        # trigger the ACT exp-table load at t=0 instead of mid-kernel
        dmy = res.tile([1, 1], F32)
        nc.vector.memset(dmy[:], 0.0)
        nc.scalar.activation(dmy[:], dmy[:], AF.Exp)

        def _guard_prefill():
            # cols [2048, 2176) of every bounce strip; needed before the
            # first readback but emitted late so startup DMAs win priority
            for b in range(BS):
                for h in range(HPC):
                    for ib in range(CS // 512):
                        g = bass.AP(tensor=pd_dram[b][h][ib], offset=KS,
                                    ap=[[TP, 512], [1, TP - KS]])
                        nc.scalar.dma_start(out=g, in_=guard_sb[:])

        def _proj_mm(acc, w_sb, xt, n_sl):
            for kt in range(NKT):
                nc.tensor.matmul(
                    acc, w_sb[:, kt, :], xt[:, kt, n_sl],
                    start=(kt == 0), stop=(kt == NKT - 1),
                )

        # ---- p projection: pT[:, t] (2 heads stacked on partitions)
        for tt in range(KS // 512):
            px = xtp.tile([128, NKT, 512], F16, tag="xtile")
            nc.scalar.dma_start(
                out=px[:],
                in_=posT.ap()[:, tt * 512 : (tt + 1) * 512].rearrange(
                    "(a p) n -> p a n", p=128
                ),
            )
            acc = ps_mm.tile([128, 512], F32, tag="mm")
            _proj_mm(acc[:], wp_sb, px, slice(0, 512))
            nc.vector.tensor_copy(pT_sb[:, tt * 512 : (tt + 1) * 512], acc[:])

        # ---- output projection partial (emitted per-b after attention)
        def _outproj(b, its):
            for it in its:
                osb = wrk.tile([128, DIN], F16, tag="osb", name="osb")
                for dh in range(DIN // 512):
                    accd = ps_mm.tile([128, 512], F32, tag="mm", name="accd")
                    nc.tensor.matmul(
                        accd[:],
                        outT_sb[:, b, it * 128 : it * 128 + 128],
                        wout_sb[:, dh * 512 : dh * 512 + 512],
                        start=True, stop=True,
                    )
                    if dh == 0:
                        nc.scalar.activation(
                            osb[:, dh * 512 : dh * 512 + 512], accd[:], AF.Copy
                        )
                    else:
                        nc.vector.tensor_copy(
                            osb[:, dh * 512 : dh * 512 + 512], accd[:]
                        )
                nc.scalar.dma_start(
                    out=out[it * 128 : it * 128 + 128, b, :], in_=osb[:],
                )

        # ---- per-batch stages
        def _proj(b):
            for tt4 in range(KS // 512):
                tt = b * (KS // 512) + tt4
                xt = xtp.tile([128, NKT, 512], F16, tag="xtile")
                nc.scalar.dma_start(
                    out=xt[:],
                    in_=xT.ap()[:, tt * 512 : (tt + 1) * 512].rearrange(
                        "(a p) n -> p a n", p=128
                    ),
                )
                # k
                acc = ps_mm.tile([128, 512], F32, tag="mm")
                _proj_mm(acc[:], wk_sb, xt, slice(0, 512))
                nc.vector.tensor_copy(kT_sb[:, tt * 512 : (tt + 1) * 512], acc[:])
                # v: [tok, hd] orientation, 4 sub-tiles of 128 toks
                for sub in range(4):
                    accv = ps_mm.tile([128, 128], F32, tag="mm")
                    n_sl = slice(sub * 128, sub * 128 + 128)
                    for kt in range(NKT):
                        nc.tensor.matmul(
                            accv[:], xt[:, kt, n_sl], wv_sb[:, kt, :],
                            start=(kt == 0), stop=(kt == NKT - 1),
                        )
                    jt = tt4 * 4 + sub
                    slot0 = b * HPC * 16 + jt
                    nc.vector.tensor_copy(
                        vaug_sb[:, slot0, 0:64], accv[:, 0:64]
                    )
                    nc.vector.tensor_copy(
                        vaug_sb[:, slot0 + 16, 0:64], accv[:, 64:128]
                    )
                # q (tokens j in [PS, KS) of this b)
                if tt4 >= PS // 512:
                    qt = b * (CS // 512) + (tt4 - PS // 512)
                    accq = ps_mm.tile([128, 512], F32, tag="mm")
                    _proj_mm(accq[:], wq_sb, xt, slice(0, 512))
                    q_sl = slice(qt * 512, (qt + 1) * 512)
                    nc.scalar.activation(
                        quT_sb[:, q_sl], accq[:], AF.Identity, bias=u_sb[:],
                    )
                    nc.vector.tensor_scalar_add(qvT_sb[:, q_sl], accq[:], v_sb[:])

        # PD psum->strip copy engine rotation (GPSIMD cannot read PSUM)
        _pd_engines = ["dve", "dve", "act", "dve", "dve",
                       "dve", "dve", "act", "dve"]
        _pd_idx = [0]

        def _pd(b, ibs=(0, 1)):
            # PD[i, t] = (q+v).p computed unshifted per i-tile, fp16 strips.
            # a-major order: the i-half consumed first is bounced first, so
            # attention for (b, ib0) unblocks after 8 strips instead of 16.
            for ib in ibs:
                for h in range(HPC):
                    hp = slice(h * 64, h * 64 + 64)
                    for s in range(4):
                        a = ib * 4 + s
                        _pd_one(b, h, hp, a, ib, s)

        def _pd_one(b, h, hp, a, ib, s):
                    tb0 = 1 - ib  # first 512-wide t-block of this half
                    width = (4 - tb0) * 512
                    o0 = max(0, 896 - 128 * a - 512 * tb0)
                    stg = stp.tile([128, 2048], F16, tag=f"stg{h}_{s}",
                                   name="stg")
                    i_sl = slice(b * CS + a * 128, b * CS + a * 128 + 128)
                    for nb in range(4 - tb0):
                        tb = tb0 + nb
                        o = o0 if nb == 0 else 0
                        accp = ps_pd.tile([128, 512], F32, tag="pd",
                                          name="accp")
                        nc.tensor.matmul(
                            accp[:, o:512], qvT_sb[hp, i_sl],
                            pT_sb[hp, tb * 512 + o : tb * 512 + 512],
                            start=True, stop=True,
                        )
                        eng = _pd_engines[_pd_idx[0] % len(_pd_engines)]
                        _pd_idx[0] += 1
                        if eng == "act":
                            nc.scalar.activation(
                                stg[:, nb * 512 + o : nb * 512 + 512],
                                accp[:, o:512], AF.Copy,
                            )
                        else:
                            nc.vector.tensor_copy(
                                stg[:, nb * 512 + o : nb * 512 + 512],
                                accp[:, o:512],
                            )
                    # one write per (b, h, a): 128 rows at s*128, in-band cols
                    dst = bass.AP(
                        tensor=pd_dram[b][h][ib],
                        offset=s * 128 * TP + tb0 * 512 + o0,
                        ap=[[TP, 128], [1, width - o0]],
                    )
                    nc.gpsimd.dma_start(out=dst, in_=stg[:, o0:width])

        # skewed plain readback: pos[i, j] = PD[i, j + 1023 - i]
        _pos_tiles = {}

        def _read2(b, ibs=(0, 1)):
            for ib in ibs:
                for h in range(HPC):
                  for s in range(4):
                    a = ib * 4 + s
                    jw = _jw(a)
                    pos = pop.tile([128, jw], F16, tag=f"pos{h}_{a}", name="pos")
                    src = bass.AP(
                        tensor=pd_dram[b][h][ib],
                        offset=s * 128 * TP + (CS - 1) - 128 * a,
                        ap=[[TP - 1, 128], [1, jw]],
                    )
                    nc.sync.dma_start(out=pos[:, 0:jw], in_=src)
                    _pos_tiles[(b, h, a)] = pos

        def _attn_ib(b, ib):
                jts = _included_jts(ib)
                pvp = [
                    ps_pv.tile([65, 512], F32, tag=f"pv{h}", name=f"pvp{h}")
                    for h in range(HPC)
                ]
                for n, jt in enumerate(jts):
                    j_sl = slice(b * KS + jt * 128, b * KS + jt * 128 + 128)
                    # clip i-columns that are fully masked for this j-tile
                    icl = max(0, jt * 128 - PS - ib * 512)
                    i_cl = slice(b * CS + ib * 512 + icl, b * CS + ib * 512 + 512)
                    for h in range(HPC):
                        hp = slice(h * 64, h * 64 + 64)
                        cn = ps_cn.tile([128, 512], F32, tag="cn")
                        subs = list(range(icl // 128, 4))
                        nc.tensor.matmul(
                            cn[:, icl:512], kT_sb[hp, j_sl], quT_sb[hp, i_cl],
                            start=True, stop=False, skip_group_check=True,
                        )
                        # add shifted position scores: transpose via identity
                        for k, s in enumerate(subs):
                            pos = _pos_tiles[(b, h, ib * 4 + s)]
                            nc.tensor.matmul(
                                cn[:, s * 128 : s * 128 + 128],
                                pos[:, jt * 128 : jt * 128 + 128],
                                id_sb[:],
                                start=False, stop=(k == len(subs) - 1),
                                skip_group_check=True,
                            )
                        atn = att.tile([128, 512], F16, tag="atn")
                        nc.scalar.activation(
                            atn[:, icl:512], cn[:, icl:512], AF.Exp, scale=SCALE
                        )
                        slot = b * HPC * 16 + h * 16 + jt
                        nc.tensor.matmul(
                            pvp[h][:, icl:512], vaug_sb[:, slot, :],
                            atn[:, icl:512],
                            start=(n == 0), stop=(n == len(jts) - 1),
                        )
                # normalize: outT[d, i] = pv[d, i] / pv[64, i].  Evacuate the
                # PSUM accumulator first so the next i-half's PV matmuls can
                # reuse the bank without waiting for the normalize chain.
                for h in range(HPC):
                    if _os.environ.get("KPVREL", "0") == "1":
                        pvs = wrk.tile([65, 512], F32, tag=f"pvs{h}")
                        nc.vector.tensor_copy(pvs[:], pvp[h][:])
                        src = pvs
                    else:
                        src = pvp[h]
                    rec = wrk.tile([1, 512], F32, tag="rec")
                    nc.vector.reciprocal(rec[:], src[64:65, :])
                    rbs = wrk.tile([64, 512], F32, tag="rbs")
                    nc.gpsimd.partition_broadcast(rbs[:], rec[:], channels=64)
                    nc.vector.tensor_mul(
                        outT_sb[h * 64 : h * 64 + 64, b, ib * 512 : ib * 512 + 512],
                        src[0:64, :], rbs[:],
                    )

        # software pipeline: attention for b interleaves with projections,
        # position-score compute and readbacks for b+1
        import os as _os
        _sched = _os.environ.get("KSCHED", "F")
        _proj(0)
        _guard_prefill()
        _pd(0)
        _read2(0)
        for b in range(BS):
            nb = b + 1
            if _sched == "A":
                if nb < BS:
                    _proj(nb)
                    _pd(nb)
                _attn_ib(b, 0)
                _attn_ib(b, 1)
                if nb < BS:
                    _read2(nb)
                _outproj(b, range(8))
            elif _sched == "B":
                _attn_ib(b, 0)
                if nb < BS:
                    _proj(nb)
                _outproj(b, range(0, 4))
                _attn_ib(b, 1)
                if nb < BS:
                    _pd(nb)
                    _read2(nb)
                _outproj(b, range(4, 8))
            elif _sched == "C":
                if nb < BS:
                    _proj(nb)
                _attn_ib(b, 0)
                if nb < BS:
                    _pd(nb)
                    _read2(nb)
                _attn_ib(b, 1)
                _outproj(b, range(8))
            elif _sched == "D":
                _attn_ib(b, 0)
                _attn_ib(b, 1)
                if nb < BS:
                    _proj(nb)
                    _pd(nb)
                    _read2(nb)
                _outproj(b, range(8))
            elif _sched == "E":
                if nb < BS:
                    _proj(nb)
                    _pd(nb)
                    _read2(nb)
                _attn_ib(b, 0)
                _attn_ib(b, 1)
                _outproj(b, range(8))
            elif _sched == "F":
                if nb < BS:
                    _proj(nb)
                    _pd(nb, (0,))
                    _read2(nb, (0,))
                _attn_ib(b, 0)
                if nb < BS:
                    _pd(nb, (1,))
                    _read2(nb, (1,))
                _attn_ib(b, 1)
                _outproj(b, range(8))
            elif _sched == "G":
                if nb < BS:
                    _proj(nb)
                    _pd(nb, (0,))
                    _read2(nb, (0,))
                _attn_ib(b, 0)
                _outproj(b, range(0, 4))
                if nb < BS:
                    _pd(nb, (1,))
                    _read2(nb, (1,))
                _attn_ib(b, 1)
                _outproj(b, range(4, 8))


def _host_prep(input_, pos_embs, memory, u, v, W_kv, W_q, W_p, W_out):
    f16 = np.float16
    xmem = np.concatenate([memory, input_], axis=0)          # (KS, BS, DIN)
    XT = np.ascontiguousarray(
        xmem.transpose(2, 1, 0).reshape(DIN, BS * KS)
    ).astype(f16)                                            # [d, b*KS+j]
    PT = np.ascontiguousarray(pos_embs.reshape(KS, DIN).T).astype(f16)
    ident = np.eye(128, dtype=np.float16)
    maps = []
    for c in range(NCORES):
        h0 = c * HPC
        sl = slice(h0 * D, h0 * D + HPC * D)
        maps.append({
            "xT": XT,
            "posT": PT,
            "wk": W_kv[:, sl].astype(f16),
            "wv": W_kv[:, H * D + h0 * D : H * D + h0 * D + HPC * D].astype(f16),
            "wq": W_q[:, sl].astype(f16),
            "wp": W_p[:, sl].astype(f16),
            "wout": W_out[sl, :].astype(f16),
            "ucol": u[h0 : h0 + HPC].reshape(HPC * D, 1).astype(np.float32),
            "vcol": v[h0 : h0 + HPC].reshape(HPC * D, 1).astype(np.float32),
            "ident": ident,
        })
    return maps


def kernel(input_, pos_embs, memory, u, v, W_kv, W_q, W_p, W_out, mask,
           _trace=False):
    if "nc" not in _CACHED:
        _CACHED["nc"] = build_nc()
    nc = _CACHED["nc"]
    args = [np.asarray(a, dtype=np.float32) for a in
            (input_, pos_embs, memory, u, v, W_kv, W_q, W_p, W_out)]
    in_maps = _host_prep(*args)
    res = run_bass_kernel_spmd(nc, in_maps, list(range(NCORES)), trace=_trace)
    total = np.zeros((CS, BS, DIN), np.float32)
    for r in res.results:
        total += np.asarray(r["out"], dtype=np.float32)
    if _trace:
        _CACHED["last_results"] = res
    return total



# revision 16
# speedup vs baseline: 1.0963x; 1.0963x over previous
"""Transformer-XL multi-head attention on 8 trn2 NeuronCores.

Sharding: tensor-parallel over heads (2 heads/core x 16 heads), all batches on
every core. Host sums the per-core partial output projections.

v3 over the v0 baseline:
  - fp8e3 (e3m4) bounce buffers for the rel-shift: position scores are
    computed pre-scaled by 1/8 (folded into qv), stored e3m4 (+-15.5 range),
    and restored by an 8*I transpose matrix -- halves the DRAM bounce+readback
    traffic at ~1e-2 worst-case rel error (gate is 2e-2).
  - software-pipelined emission: attention (ACT-paced exp) is woven with
    projection / position-score PE work units for the next batch so the PE
    never head-of-line blocks; PV for j-tile n is emitted after scores for
    j-tile n+1.
  - all DMA dispatches routed off the ACT queue (SP/Pool) so exp never
    stalls behind a 667ns DMA decode.

Per-core pipeline (fp16 operands, fp32 PSUM):
  1. Projections from host-prepped fp16 XT/posT: kT, quT, qvT(/8), pT in
     [head*d, token]; V in [token, head*d] + ones column.
  2. PD[i,t] = ((q_i+v)/8).p_t per i-tile, fp8e3 strips -> DRAM bounce with
     guard band (t >= 2048) prefilled at -15.5.
  3. Skewed affine DMA readback pos[i, j] = PD[i, j+1023-i]: rel-shift +
     causal mask for free.
  4. Content scores in [j, i]; pos added into the same PSUM group via
     (8*I)-rhs transpose matmuls; ACT exp -> fp16 attn.
  5. PV with V augmented by ones (row 64 = denominator), DVE reciprocal +
     GPSIMD broadcast normalize, fp16 output projection partials summed on
     host.
"""

import contextlib
import os as _os

import numpy as np

import concourse.bass as bass
from concourse import bacc
import concourse.mybir as mybir
import concourse.tile as tile
from concourse.bass_utils import run_bass_kernel_spmd

F8E3 = mybir.dt.float8e3
F16 = mybir.dt.float16
F32 = mybir.dt.float32
AF = mybir.ActivationFunctionType
ALU = mybir.AluOpType

CS, PS, BS, DIN, H, D = 1024, 1024, 4, 1024, 16, 64
KS = CS + PS                 # 2048 keys
NCORES = 8
HPC = H // NCORES            # 2 heads per core
TOK = KS * BS                # 8192 kv tokens (b-major: tok = b*KS + j)
QTOK = CS * BS               # 4096 q tokens  (tok = b*CS + i)
TP = KS + 128                # PD row length incl. guard band
GUARD = -15.5                # e3m4 max-negative; after *8 -> -124, exp -> ~0
SCALE = 1.0 / D ** 0.5
PSC = 8.0                    # pos pre-scale (e3m4 range +-15.5)
NKT = DIN // 128             # 8 contraction tiles of 128

_CACHED = {}


def _included_jts(ib):
    """j-tiles (128 wide) with any unmasked element for i-half ib (512 wide)."""
    i_hi = ib * 512 + 511
    return [jt for jt in range(KS // 128) if jt * 128 <= i_hi + PS]


def _jw(a):
    """valid j-width for i-tile a (i in [128a, 128a+128)): j <= i + PS."""
    return min(KS, PS + (a + 1) * 128)


def build_nc():
    nc = bacc.Bacc()
    xT = nc.declare_dram_parameter("xT", [DIN, TOK], F16, isOutput=False)
    posT = nc.declare_dram_parameter("posT", [DIN, KS], F16, isOutput=False)
    wk = nc.declare_dram_parameter("wk", [DIN, 128], F16, isOutput=False)
    wv = nc.declare_dram_parameter("wv", [DIN, 128], F16, isOutput=False)
    wq = nc.declare_dram_parameter("wq", [DIN, 128], F16, isOutput=False)
    wp = nc.declare_dram_parameter("wp", [DIN, 128], F16, isOutput=False)
    wout = nc.declare_dram_parameter("wout", [128, DIN], F16, isOutput=False)
    ucol = nc.declare_dram_parameter("ucol", [128, 1], F32, isOutput=False)
    vcol = nc.declare_dram_parameter("vcol", [128, 1], F32, isOutput=False)
    ident = nc.declare_dram_parameter("ident", [128, 128], F8E3, isOutput=False)
    out = nc.declare_dram_parameter("out", [CS, BS, DIN], F16, isOutput=True)

    # PD bounce buffers, one per (b, h, i-half) for fine-grained deps
    pd_dram = [
        [[nc.dram_tensor(f"pd_{b}_{h}_{ib}", [512, TP], F8E3)
          for ib in range(CS // 512)] for h in range(HPC)]
        for b in range(BS)
    ]

    with tile.TileContext(nc) as tc:
        _body(nc, tc, xT, posT, wk, wv, wq, wp, wout, ucol, vcol, ident, out,
              pd_dram)
    nc.compile()
    return nc


def _body(nc, tc, xT, posT, wk, wv, wq, wp, wout, ucol, vcol, ident, out,
          pd_dram):
    ctx = contextlib.ExitStack()
    with ctx:
        res = ctx.enter_context(tc.tile_pool(name="res", bufs=1))
        xtp = ctx.enter_context(tc.tile_pool(
            name="xtp", bufs=int(_os.environ.get("KXTP", "3"))))
        stp = ctx.enter_context(tc.tile_pool(name="stp", bufs=1))
        pop = ctx.enter_context(tc.tile_pool(
            name="pop", bufs=int(_os.environ.get("KPOP", "2"))))
        att = ctx.enter_context(tc.tile_pool(
            name="att", bufs=int(_os.environ.get("KATT", "6"))))
        wrk = ctx.enter_context(tc.tile_pool(
            name="wrk", bufs=int(_os.environ.get("KWRK", "3"))))
        _pp = [int(c) for c in _os.environ.get("KPSUM", "2222")]
        ps_mm = ctx.enter_context(tc.tile_pool(name="ps_mm", bufs=_pp[0], space="PSUM"))
        ps_pd = ctx.enter_context(tc.tile_pool(name="ps_pd", bufs=_pp[1], space="PSUM"))
        ps_cn = ctx.enter_context(tc.tile_pool(name="ps_cn", bufs=_pp[2], space="PSUM"))
        ps_pv = ctx.enter_context(tc.tile_pool(name="ps_pv", bufs=_pp[3], space="PSUM"))

        # ---- resident tiles
        kT_sb = res.tile([128, TOK], F16)         # [(h,d), b*KS+j]
        quT_sb = res.tile([128, QTOK], F16)       # [(h,d), b*CS+i]
        qvT_sb = res.tile([128, QTOK], F16)       # (q+v)/8 for the e3m4 bounce
        pT_sb = res.tile([128, KS], F16)
        # V + ones column: slot index = b*HPC*16 + h*16 + jt
        vaug_sb = res.tile([128, BS * HPC * 16, 65], F16)
        outT_sb = res.tile([128, BS, CS], F16)    # [(h,d), b, i]
        wk_sb = res.tile([128, NKT, 128], F16)
        wv_sb = res.tile([128, NKT, 128], F16)
        wq_sb = res.tile([128, NKT, 128], F16)
        wp_sb = res.tile([128, NKT, 128], F16)
        wout_sb = res.tile([128, DIN], F16)
        u_sb = res.tile([128, 1], F32)
        v_sb = res.tile([128, 1], F32)            # host pre-scaled v/8
        id_sb = res.tile([128, 128], F8E3)        # 8 * I

        nc.sync.dma_start(out=wp_sb[:], in_=wp.ap().rearrange("(a p) n -> p a n", p=128))
        nc.sync.dma_start(out=wk_sb[:], in_=wk.ap().rearrange("(a p) n -> p a n", p=128))
        nc.sync.dma_start(out=wv_sb[:], in_=wv.ap().rearrange("(a p) n -> p a n", p=128))
        nc.sync.dma_start(out=wq_sb[:], in_=wq.ap().rearrange("(a p) n -> p a n", p=128))
        nc.sync.dma_start(out=wout_sb[:], in_=wout[:, :])
        nc.sync.dma_start(out=u_sb[:], in_=ucol[:, :])
        nc.sync.dma_start(out=v_sb[:], in_=vcol[:, :])
        nc.sync.dma_start(out=id_sb[:], in_=ident[:, :])
        nc.vector.memset(vaug_sb[:, :, 64:65], 1.0)
        guard_sb = res.tile([128, 512], F8E3)
        nc.vector.memset(guard_sb[:], GUARD)
        # trigger the ACT exp-table load at t=0 instead of mid-kernel
        dmy = res.tile([1, 1], F32)
        nc.vector.memset(dmy[:], 0.0)
        nc.scalar.activation(dmy[:], dmy[:], AF.Exp)

        def _guard_prefill():
            # cols [2048, 2176) of every bounce strip; needed before the
            # first readback but emitted late so startup DMAs win priority
            for b in range(BS):
                for h in range(HPC):
                    for ib in range(CS // 512):
                        g = bass.AP(tensor=pd_dram[b][h][ib], offset=KS,
                                    ap=[[TP, 512], [1, TP - KS]])
                        nc.sync.dma_start(out=g, in_=guard_sb[:])

        # psum->sbuf copy engine rotation (ACT also carries exp; DVE the rest)
        _cp_engines = ["dve", "act", "dve", "act", "dve", "act", "dve"]
        _cp_idx = [0]

        def _cp(dst, src):
            eng = _cp_engines[_cp_idx[0] % len(_cp_engines)]
            _cp_idx[0] += 1
            if eng == "act":
                nc.scalar.activation(dst, src, AF.Copy)
            else:
                nc.vector.tensor_copy(dst, src)

        def _proj_mm(acc, w_sb, xt, n_sl):
            for kt in range(NKT):
                nc.tensor.matmul(
                    acc, w_sb[:, kt, :], xt[:, kt, n_sl],
                    start=(kt == 0), stop=(kt == NKT - 1),
                )

        # ---- p projection: pT[:, t] (2 heads stacked on partitions)
        def _pproj():
            for tt in range(KS // 512):
                px = xtp.tile([128, NKT, 512], F16, tag="xtile")
                nc.sync.dma_start(
                    out=px[:],
                    in_=posT.ap()[:, tt * 512 : (tt + 1) * 512].rearrange(
                        "(a p) n -> p a n", p=128),
                )
                acc = ps_mm.tile([128, 512], F32, tag="mm")
                _proj_mm(acc[:], wp_sb, px, slice(0, 512))
                nc.scalar.activation(
                    pT_sb[:, tt * 512 : (tt + 1) * 512], acc[:], AF.Copy)

        # ---- output projection partial
        def _outproj(b, its):
            for it in its:
                osb = wrk.tile([128, DIN], F16, tag="osb", name="osb")
                for dh in range(DIN // 512):
                    accd = ps_mm.tile([128, 512], F32, tag="mm", name="accd")
                    nc.tensor.matmul(
                        accd[:],
                        outT_sb[:, b, it * 128 : it * 128 + 128],
                        wout_sb[:, dh * 512 : dh * 512 + 512],
                        start=True, stop=True,
                    )
                    _cp(osb[:, dh * 512 : dh * 512 + 512], accd[:])
                nc.sync.dma_start(
                    out=out[it * 128 : it * 128 + 128, b, :], in_=osb[:],
                )

        # ---- per-batch projections (one unit per 512-token tile)
        def _proj_tt(b, tt4):
            tt = b * (KS // 512) + tt4
            xt = xtp.tile([128, NKT, 512], F16, tag="xtile")
            nc.sync.dma_start(
                out=xt[:],
                in_=xT.ap()[:, tt * 512 : (tt + 1) * 512].rearrange(
                    "(a p) n -> p a n", p=128),
            )
            # k
            acc = ps_mm.tile([128, 512], F32, tag="mm")
            _proj_mm(acc[:], wk_sb, xt, slice(0, 512))
            _cp(kT_sb[:, tt * 512 : (tt + 1) * 512], acc[:])
            # v: [tok, hd] orientation, 4 sub-tiles of 128 toks
            for sub in range(4):
                accv = ps_mm.tile([128, 128], F32, tag="mm")
                n_sl = slice(sub * 128, sub * 128 + 128)
                for kt in range(NKT):
                    nc.tensor.matmul(
                        accv[:], xt[:, kt, n_sl], wv_sb[:, kt, :],
                        start=(kt == 0), stop=(kt == NKT - 1),
                    )
                jt = tt4 * 4 + sub
                slot0 = b * HPC * 16 + jt
                _cp(vaug_sb[:, slot0, 0:64], accv[:, 0:64])
                _cp(vaug_sb[:, slot0 + 16, 0:64], accv[:, 64:128])
            # q (tokens j in [PS, KS) of this b)
            if tt4 >= PS // 512:
                qt = b * (CS // 512) + (tt4 - PS // 512)
                accq = ps_mm.tile([128, 512], F32, tag="mm")
                _proj_mm(accq[:], wq_sb, xt, slice(0, 512))
                q_sl = slice(qt * 512, (qt + 1) * 512)
                nc.scalar.activation(
                    quT_sb[:, q_sl], accq[:], AF.Identity, bias=u_sb[:],
                )
                # (q + v)/8 for the e3m4 bounce (v_sb is host-prescaled v/8)
                nc.vector.tensor_scalar(
                    qvT_sb[:, q_sl], accq[:], 1.0 / PSC, v_sb[:],
                    ALU.mult, ALU.add,
                )

        def _proj_units(b):
            return [lambda tt4=tt4: _proj_tt(b, tt4) for tt4 in range(4)]

        # ---- position scores PD[i, t]/8 and e3m4 strips
        def _pd_one(b, h, hp, a, ib, s):
            tb0 = 1 - ib  # first 512-wide t-block of this half
            width = (4 - tb0) * 512
            o0 = max(0, 896 - 128 * a - 512 * tb0)
            stg = stp.tile([128, 2048], F8E3, tag=f"stg{h}_{s}", name="stg")
            i_sl = slice(b * CS + a * 128, b * CS + a * 128 + 128)
            for nb in range(4 - tb0):
                tb = tb0 + nb
                o = o0 if nb == 0 else 0
                accp = ps_pd.tile([128, 512], F32, tag="pd", name="accp")
                nc.tensor.matmul(
                    accp[:, o:512], qvT_sb[hp, i_sl],
                    pT_sb[hp, tb * 512 + o : tb * 512 + 512],
                    start=True, stop=True,
                )
                _cp(stg[:, nb * 512 + o : nb * 512 + 512], accp[:, o:512])
            dst = bass.AP(
                tensor=pd_dram[b][h][ib],
                offset=s * 128 * TP + tb0 * 512 + o0,
                ap=[[TP, 128], [1, width - o0]],
            )
            nc.gpsimd.dma_start(out=dst, in_=stg[:, o0:width])

        def _pd_units(b, ib):
            us = []
            for h in range(HPC):
                hp = slice(h * 64, h * 64 + 64)
                for s in range(4):
                    a = ib * 4 + s
                    us.append(lambda b=b, h=h, hp=hp, a=a, ib=ib, s=s:
                              _pd_one(b, h, hp, a, ib, s))
            return us

        def _out_units(b, its):
            return [lambda it=it: _outproj(b, (it,)) for it in its]

        # skewed plain readback: pos[i, j] = PD[i, j + 1023 - i]
        _pos_tiles = {}

        def _read2(b, ibs=(0, 1)):
            for ib in ibs:
                for h in range(HPC):
                    for s in range(4):
                        a = ib * 4 + s
                        jw = _jw(a)
                        pos = pop.tile([128, 2048], F8E3, tag=f"pos{h}_{a}",
                                       name="pos")
                        src = bass.AP(
                            tensor=pd_dram[b][h][ib],
                            offset=s * 128 * TP + (CS - 1) - 128 * a,
                            ap=[[TP - 1, 128], [1, jw]],
                        )
                        nc.sync.dma_start(out=pos[:, 0:jw], in_=src)
                        _pos_tiles[(b, h, a)] = pos

        # ---- attention scores + exp for one (b, ib, h, jt)
        def _attn_scores(b, ib, h, n, jt, atns):
            hp = slice(h * 64, h * 64 + 64)
            icl = max(0, jt * 128 - PS - ib * 512)
            j_sl = slice(b * KS + jt * 128, b * KS + jt * 128 + 128)
            i_cl = slice(b * CS + ib * 512 + icl, b * CS + ib * 512 + 512)
            cn = ps_cn.tile([128, 512], F32, tag="cn")
            nc.tensor.matmul(
                cn[:, icl:512], kT_sb[hp, j_sl], quT_sb[hp, i_cl],
                start=True, stop=False, skip_group_check=True,
            )
            subs = list(range(icl // 128, 4))
            for k, s in enumerate(subs):
                pos = _pos_tiles[(b, h, ib * 4 + s)]
                nc.tensor.matmul(
                    cn[:, s * 128 : s * 128 + 128],
                    pos[:, jt * 128 : jt * 128 + 128],
                    id_sb[:],
                    start=False, stop=(k == len(subs) - 1),
                    skip_group_check=True,
                )
            atn = att.tile([128, 512], F16, tag=f"atn{h}")
            atns[n] = (atn, icl, jt)
            nc.scalar.activation(
                atn[:, icl:512], cn[:, icl:512], AF.Exp, scale=SCALE,
            )

        def _attn_pv(b, ib, h, n, atns, pvp, njt):
            atn, icl, jt = atns.pop(n)
            slot = b * HPC * 16 + h * 16 + jt
            nc.tensor.matmul(
                pvp[:, icl:512], vaug_sb[:, slot, :], atn[:, icl:512],
                start=(n == 0), stop=(n == njt - 1),
                skip_group_check=True,
            )

        def _attn_norm(b, ib, h, pvp):
            rec = wrk.tile([1, 512], F32, tag="rec")
            nc.vector.reciprocal(rec[:], pvp[64:65, :])
            rbs = wrk.tile([64, 512], F32, tag="rbs")
            nc.gpsimd.partition_broadcast(rbs[:], rec[:], channels=64)
            nc.vector.tensor_mul(
                outT_sb[h * 64 : h * 64 + 64, b, ib * 512 : ib * 512 + 512],
                pvp[0:64, :], rbs[:],
            )

        def _attn_ib(b, ib, fill):
            """Heads sequential; PV for j-tile n emitted after scores for
            n+1; `fill` PE work units woven between j-tiles."""
            jts = _included_jts(ib)
            njt = len(jts)
            fi = 0
            nf = len(fill)
            nu = HPC * njt
            u = 0
            for h in range(HPC):
                pvp = ps_pv.tile([65, 512], F32, tag="pv", name="pvp")
                atns = {}
                for n, jt in enumerate(jts):
                    _attn_scores(b, ib, h, n, jt, atns)
                    if n > 0:
                        _attn_pv(b, ib, h, n - 1, atns, pvp, njt)
                    u += 1
                    want = nf * u // nu
                    while fi < want:
                        fill[fi]()
                        fi += 1
                _attn_pv(b, ib, h, njt - 1, atns, pvp, njt)
                _attn_norm(b, ib, h, pvp)
            while fi < nf:
                fill[fi]()
                fi += 1

        # ---- software pipeline over b
        _pproj()
        for tt4 in range(4):
            _proj_tt(0, tt4)
        _guard_prefill()
        for u_ in _pd_units(0, 0):
            u_()
        _read2(0, (0,))
        for b in range(BS):
            nb = b + 1
            fill0 = []
            if b == 0:
                fill0 += _pd_units(0, 1)
            if nb < BS:
                pu = _proj_units(nb)
                du = _pd_units(nb, 0)
                # q-proj for i-half 0 comes from tt4=2: emit P0..P2 before
                # the pd units that read qvT, P3 afterwards.
                fill0 += pu[:3] + du[:4] + pu[3:] + du[4:]
            _attn_ib(b, 0, fill0)
            if b == 0:
                _read2(0, (1,))
            if nb < BS:
                _read2(nb, (0,))
            fill1 = []
            if nb < BS:
                fill1 += _pd_units(nb, 1)
            fill1 += _out_units(b, range(0, 4))
            _attn_ib(b, 1, fill1)
            if nb < BS:
                _read2(nb, (1,))
            _outproj(b, range(4, 8))


def _host_prep(input_, pos_embs, memory, u, v, W_kv, W_q, W_p, W_out):
    import ml_dtypes
    f16 = np.float16
    xmem = np.concatenate([memory, input_], axis=0)          # (KS, BS, DIN)
    XT = np.ascontiguousarray(
        xmem.transpose(2, 1, 0).reshape(DIN, BS * KS)
    ).astype(f16)                                            # [d, b*KS+j]
    PT = np.ascontiguousarray(pos_embs.reshape(KS, DIN).T).astype(f16)
    ident = (np.eye(128, dtype=np.float32) * PSC).astype(ml_dtypes.float8_e3m4)
    maps = []
    for c in range(NCORES):
        h0 = c * HPC
        sl = slice(h0 * D, h0 * D + HPC * D)
        maps.append({
            "xT": XT,
            "posT": PT,
            "wk": W_kv[:, sl].astype(f16),
            "wv": W_kv[:, H * D + h0 * D : H * D + h0 * D + HPC * D].astype(f16),
            "wq": W_q[:, sl].astype(f16),
            "wp": W_p[:, sl].astype(f16),
            "wout": W_out[sl, :].astype(f16),
            "ucol": u[h0 : h0 + HPC].reshape(HPC * D, 1).astype(np.float32),
            "vcol": (v[h0 : h0 + HPC].reshape(HPC * D, 1) / PSC
                     ).astype(np.float32),
            "ident": ident,
        })
    return maps


def kernel(input_, pos_embs, memory, u, v, W_kv, W_q, W_p, W_out, mask,
           _trace=False):
    if "nc" not in _CACHED:
        _CACHED["nc"] = build_nc()
    nc = _CACHED["nc"]
    args = [np.asarray(a, dtype=np.float32) for a in
            (input_, pos_embs, memory, u, v, W_kv, W_q, W_p, W_out)]
    in_maps = _host_prep(*args)
    res = run_bass_kernel_spmd(nc, in_maps, list(range(NCORES)), trace=_trace)
    total = np.zeros((CS, BS, DIN), np.float32)
    for r in res.results:
        total += np.asarray(r["out"], dtype=np.float32)
    if _trace:
        _CACHED["last_results"] = res
    return total
